# revision 29
# baseline (speedup 1.0000x reference)
"""Trainium2 Bass kernel for nn_Attention_46273977647279.

Multi-head attention (B=4, S=2048, D=512, H=8, DK=64, no 1/sqrt(dk) scale)
returning (out, attn).  attn is [B, H, Sq, Sk] fp32 = 512 MB, which dominates
HBM traffic (target_regime=memory).

Sharding: 8 cores = (batch b in 0..3) x (head-group g in 0..1, 4 heads each).
Each core computes its 4 heads of one batch:
  - host stages transposed activations/weights (layout only, no FLOPs)
  - device: QT/KT = [d, q] projections, V = [k, d] (+ ones column)
  - per (head, q-half): scores^T tiles [k,q] via one matmul each (d=64
    contraction), a SINGLE scalar-engine exp pass (mask folded in as the
    per-partition activation bias), AV matmul accumulates blended^T with the
    softmax denominators arriving free via the ones column of V, then DVE
    normalizes exp tiles in-place and DMAs attn^T [h, k, q] to HBM.
  - wo partial product [q, 512] at the end; host sums the two head-group
    partials and adds wo_b.
Host returns attn as a zero-copy transposed view of the stacked [.., k, q]
shards.
"""

import numpy as np

import concourse.bass as bass
import concourse.bacc as bacc
import concourse.mybir as mybir
import concourse.tile as tile
from concourse.bass_utils import run_bass_kernel_spmd

B, S, DE, DM, H = 4, 2048, 512, 512, 8
DK = DM // H          # 64
HG = 2                # head groups (tensor-parallel dim)
HPG = H // HG         # 4 heads per group
DG = HPG * DK         # 256 projection rows per group
P = 128               # partitions
NQT = S // P          # 16 q tiles of 128
NKT = S // P          # 16 k tiles of 128
QHALF = 1024          # q processed per half-head (bounds live exp SBUF)
NEG = -1e9

F32 = mybir.dt.float32
F32R = mybir.dt.float32r
AF = mybir.ActivationFunctionType


def _r(ap):
    """Reinterpret an fp32 AP as float32r for the fast PE matmul path."""
    return ap.bitcast(F32R)

_CACHE = {}


def build_bass():
    nc = bacc.Bacc("TRN2", target_bir_lowering=False, debug=False, num_devices=8)

    # ---- per-core inputs (host-staged layouts) ----
    qeT = nc.dram_tensor("qeT", [DE, S], F32, kind="ExternalInput")     # qe[b].T
    ieT = nc.dram_tensor("ieT", [DE, S], F32, kind="ExternalInput")     # ie[b].T
    wqT = nc.dram_tensor("wqT", [DE, DG], F32, kind="ExternalInput")    # wq[g].T
    wkT = nc.dram_tensor("wkT", [DE, DG], F32, kind="ExternalInput")
    wvT = nc.dram_tensor("wvT", [DE, DG], F32, kind="ExternalInput")
    woT = nc.dram_tensor("woT", [DG, DE], F32, kind="ExternalInput")    # wo[:, g cols].T
    wqb = nc.dram_tensor("wqb", [P, DG // P], F32, kind="ExternalInput")  # [128, 2]
    wkb = nc.dram_tensor("wkb", [P, DG // P], F32, kind="ExternalInput")
    wvb = nc.dram_tensor("wvb", [1, DG], F32, kind="ExternalInput")
    maskb = nc.dram_tensor("maskb", [P, NKT], F32, kind="ExternalInput")  # additive mask

    # ---- per-core outputs ----
    attnT = nc.dram_tensor("attnT", [HPG, S, S], F32, kind="ExternalOutput")  # [h, k, q]
    outp = nc.dram_tensor("outp", [S, DE], F32, kind="ExternalOutput")        # wo partial

    ET = DE // P  # 4 e tiles

    with tile.TileContext(nc) as tc, nc.allow_low_precision(
        reason="float32r matmul operands; fp32 accumulation in PSUM"
    ):
        # ---------- persistent SBUF ----------
        with (
            tc.tile_pool(name="persist", bufs=1) as persist,
            tc.tile_pool(name="small", bufs=1) as small,
            tc.tile_pool(name="psum", bufs=2, space="PSUM") as psum,
        ):
            QT_sb = persist.tile([P, DG // P, S], F32R)    # [128, 2, 2048] d-major
            KT_sb = persist.tile([P, DG // P, S], F32R)
            V_sb = persist.tile([P, NKT, HPG * (DK + 1)], F32R)  # [128, 16, 260]
            concatT = persist.tile([P, DG // P, S], F32R)  # normalized blended^T
            woT_sb = persist.tile([P, DG // P, DE], F32R)  # [128, 2, 512]
            maskb_sb = small.tile([P, NKT], F32)
            ones_sb = small.tile([1, P], F32)
            ones_f32 = small.tile([P, P], F32)  # scratch for rounding
            bvB_sb = small.tile([P, DG], F32)
            wqb_sb = small.tile([P, DG // P], F32)
            wkb_sb = small.tile([P, DG // P], F32)

            nc.vector.memset(ones_f32, 1.0)
            warm = small.tile([1, 2], F32)
            nc.vector.memset(warm, 0.0)
            nc.scalar.activation(warm, warm, AF.Exp, bias=0.0, scale=1.0)
            nc.vector.tensor_copy(ones_sb, ones_f32[0:1, :])
            # ones column for the AV sum trick (rounded f32r via DVE copy)
            nc.vector.tensor_copy(
                V_sb.rearrange("p k (h c) -> p k h c", c=DK + 1)[:, :, :, DK:DK + 1],
                ones_f32[:, 0:NKT * HPG].rearrange(
                    "p (k h c) -> p k h c", h=HPG, c=1),
            )

            # ---------- phase 1: projections ----------
            with tc.tile_pool(name="stage", bufs=1) as stage:
                qeT_sb = stage.tile([P, ET, S], F32R)   # [128, 4, 2048]
                ieT_sb = stage.tile([P, ET, S], F32R)
                wqT_sb = stage.tile([P, ET, DG], F32R)  # [128, 4, 256]
                wkT_sb = stage.tile([P, ET, DG], F32R)
                wvT_sb = stage.tile([P, ET, DG], F32R)
                # small weight tensors first so projection matmuls only
                # gate on the big embed tiles they actually consume
                nc.sync.dma_start(
                    out=wkT_sb, in_=wkT.rearrange("(t p) d -> p t d", p=P).bitcast(F32R)
                )
                nc.sync.dma_start(
                    out=wqT_sb, in_=wqT.rearrange("(t p) d -> p t d", p=P).bitcast(F32R)
                )
                qeT_r = qeT.rearrange("(t p) q -> p t q", p=P)
                ieT_r = ieT.rearrange("(t p) q -> p t q", p=P)
                nc.sync.dma_start(out=wkb_sb, in_=wkb[:, :])
                nc.sync.dma_start(out=wqb_sb, in_=wqb[:, :])
                for et in range(ET):
                    nc.sync.dma_start(out=ieT_sb[:, et], in_=ieT_r[:, et].bitcast(F32R))
                for et in range(ET):
                    nc.sync.dma_start(out=qeT_sb[:, et], in_=qeT_r[:, et].bitcast(F32R))
                nc.sync.dma_start(
                    out=wvT_sb, in_=wvT.rearrange("(t p) d -> p t d", p=P).bitcast(F32R)
                )
                nc.sync.dma_start(out=maskb_sb, in_=maskb[:, :])
                wvb_ap = wvb[0:1, :]
                bvB_bcast = bass.AP(
                    tensor=wvb_ap.tensor, offset=wvb_ap.offset, ap=[[0, P], [1, DG]]
                )
                nc.sync.dma_start(out=bvB_sb, in_=bvB_bcast)

                # Projections, ordered along the attention critical path:
                # head0/qh0 needs KT[dt0, all qc] + QT[dt0, qc0-1] first, then
                # V tiles in kc order; the rest can trail.
                def qk_chunk(wsb, xsb, bias_sb, dst, dt, qc):
                    pj = psum.tile([P, 512], F32, tag="sc", name=f"pj_{dt}_{qc}")
                    for et in range(ET):
                        nc.tensor.matmul(
                            pj,
                            _r(wsb[:, et, dt * P:(dt + 1) * P]),
                            _r(xsb[:, et, qc * 512:(qc + 1) * 512]),
                            start=(et == 0),
                            stop=(et == ET - 1),
                        )
                    nc.vector.tensor_scalar_add(
                        dst[:, dt, qc * 512:(qc + 1) * 512], pj,
                        bias_sb[:, dt:dt + 1],
                    )

                def v_chunk(kc):
                    pv = psum.tile([P, DG], F32, tag="sc", name=f"pv_{kc}")
                    for et in range(ET):
                        nc.tensor.matmul(
                            pv,
                            _r(ieT_sb[:, et, kc * P:(kc + 1) * P]),
                            _r(wvT_sb[:, et, :]),
                            start=(et == 0),
                            stop=(et == ET - 1),
                        )
                    nc.vector.tensor_add(
                        V_sb[:, kc].rearrange("p (h c) -> p h c", h=HPG)[:, :, 0:DK],
                        pv.rearrange("p (h c) -> p h c", h=HPG),
                        bvB_sb.rearrange("p (h c) -> p h c", h=HPG),
                    )

                QK = {"q": (wqT_sb, qeT_sb, wqb_sb, QT_sb),
                      "k": (wkT_sb, ieT_sb, wkb_sb, KT_sb)}
                for qc in range(4):
                    qk_chunk(*QK["k"], 0, qc)
                for qc in range(2):
                    qk_chunk(*QK["q"], 0, qc)
                for kc in range(NKT):
                    v_chunk(kc)
                for qc in range(4):
                    qk_chunk(*QK["k"], 1, qc)
                for qc in range(2):
                    qk_chunk(*QK["q"], 1, qc)
                for dt in range(2):
                    for qc in (2, 3):
                        qk_chunk(*QK["q"], dt, qc)

            # ---------- phase 2: attention ----------
            with (
                tc.tile_pool(name="expp", bufs=28) as expp,
                tc.tile_pool(name="rzp", bufs=2) as rzp,
                tc.tile_pool(name="outsb", bufs=3) as outsb,
            ):
                GP_TILES = {2, 4, 7, 9, 12, 14}   # normalize tiles -> gpsimd

                def attn_block(h, q0, qlen):
                    """scores^T -> exp -> AV(+Z) -> normalize -> attn DMA for
                    head h over q columns [q0, q0+qlen)."""
                    po = (h % 2) * DK     # partition offset of head's d rows
                    dt = h // 2           # which d-tile
                    nch = qlen // 512
                    av = []
                    for c in range(nch):
                        av_t = psum.tile([P, 512], F32, tag="av", bufs=3,
                                         name=f"av_{h}_{q0}_{c}")
                        av.append(av_t)
                    exp_tiles = []
                    for t in range(NKT):
                        sc = psum.tile([P, qlen], F32, tag="sc", name=f"sc_{h}_{q0}_{t}")
                        for c in range(nch):
                            nc.tensor.matmul(
                                sc[:, c * 512:(c + 1) * 512],
                                _r(KT_sb[po:po + DK, dt, t * P:(t + 1) * P]),
                                _r(QT_sb[po:po + DK, dt,
                                         q0 + c * 512:q0 + (c + 1) * 512]),
                                start=True,
                                stop=True,
                            )
                        e = expp.tile([P, qlen], F32R, tag="exp",
                                      name=f"e_{h}_{q0}_{t}")
                        nc.scalar.activation(
                            e, sc, AF.Exp, bias=maskb_sb[:, t:t + 1], scale=1.0
                        )
                        exp_tiles.append(e)
                        for c in range(nch):
                            nc.tensor.matmul(
                                av[c][0:DK + 1, :],
                                _r(V_sb[:, t, h * (DK + 1):(h + 1) * (DK + 1)]),
                                _r(e[:, c * 512:(c + 1) * 512]),
                                start=(t == 0),
                                stop=(t == NKT - 1),
                            )
                    # softmax denominators -> reciprocal -> broadcast
                    rz = rzp.tile([P, qlen], F32, tag="rz", name=f"rz_{h}_{q0}")
                    rrow = rzp.tile([1, qlen], F32, tag="rrow", name=f"rr_{h}_{q0}")
                    for c in range(nch):
                        nc.vector.reciprocal(
                            rrow[0:1, c * 512:(c + 1) * 512], av[c][DK:DK + 1, :]
                        )
                        bc = psum.tile([P, 512], F32, tag="bc", bufs=1,
                                       name=f"bc_{h}_{q0}_{c}")
                        nc.tensor.matmul(
                            bc,
                            ones_sb[0:1, :],
                            rrow[0:1, c * 512:(c + 1) * 512],
                            start=True,
                            stop=True,
                        )
                        nc.vector.tensor_copy(rz[:, c * 512:(c + 1) * 512], bc)
                    # normalized blended^T into concatT
                    for c in range(nch):
                        nc.vector.tensor_mul(
                            concatT[po:po + DK, dt,
                                    q0 + c * 512:q0 + (c + 1) * 512],
                            av[c][0:DK, :],
                            rz[0:DK, c * 512:(c + 1) * 512],
                        )
                    # normalize attn tiles in place and write out
                    for t in range(NKT):
                        e = exp_tiles[t]
                        eng = nc.gpsimd if t in GP_TILES else nc.vector
                        eng.tensor_mul(e, e, rz)
                        nc.sync.dma_start(
                            out=attnT[h, t * P:(t + 1) * P,
                                      q0:q0 + qlen].bitcast(F32R),
                            in_=e,
                        )

                def wo_block(q0, qlen):
                    for qc in range(q0 // P, (q0 + qlen) // P):
                        po_t = psum.tile([P, DE], F32, tag="av", bufs=3,
                                         name=f"po_{qc}")
                        for dt in range(DG // P):
                            nc.tensor.matmul(
                                po_t,
                                _r(concatT[:, dt, qc * P:(qc + 1) * P]),
                                _r(woT_sb[:, dt, :]),
                                start=(dt == 0),
                                stop=(dt == DG // P - 1),
                            )
                        o = outsb.tile([P, DE], F32, tag="o", name=f"o_{qc}")
                        nc.scalar.copy(o, po_t)
                        nc.sync.dma_start(out=outp[qc * P:(qc + 1) * P, :], in_=o)

                nc.sync.dma_start(
                    out=woT_sb,
                    in_=woT.rearrange("(t p) e -> p t e", p=P).bitcast(F32R),
                )
                # first head split into q-quarters so the attn DMA stream
                # starts as early as possible after the input load
                attn_block(0, 0, 512)
                attn_block(0, 512, 512)
                for h in range(1, HPG):
                    attn_block(h, 0, QHALF)
                wo_block(0, QHALF)
                for h in range(HPG):
                    attn_block(h, QHALF, QHALF)
                wo_block(QHALF, QHALF)

    nc.compile()
    return nc


def _prep_inputs(input_embeds, query_embeds, mask, wq_w, wq_b, wk_w, wk_b,
                 wv_w, wv_b, wo_w):
    """Stage per-core input maps (slicing/transposition only)."""
    f = np.float32
    in_maps = []
    for core in range(8):
        b, g = core // HG, core % HG
        sl = slice(g * DG, (g + 1) * DG)
        mb = np.where(mask[b] > 0.5, 0.0, NEG).astype(f)
        in_maps.append({
            "qeT": np.ascontiguousarray(query_embeds[b].T, dtype=f),
            "ieT": np.ascontiguousarray(input_embeds[b].T, dtype=f),
            "wqT": np.ascontiguousarray(wq_w[sl].T, dtype=f),
            "wkT": np.ascontiguousarray(wk_w[sl].T, dtype=f),
            "wvT": np.ascontiguousarray(wv_w[sl].T, dtype=f),
            "woT": np.ascontiguousarray(wo_w[:, sl].T, dtype=f),
            "wqb": np.ascontiguousarray(wq_b[sl].reshape(DG // P, P).T, dtype=f),
            "wkb": np.ascontiguousarray(wk_b[sl].reshape(DG // P, P).T, dtype=f),
            "wvb": np.ascontiguousarray(wv_b[sl].reshape(1, DG), dtype=f),
            "maskb": np.ascontiguousarray(mb.reshape(NKT, P).T, dtype=f),
        })
    return in_maps


def kernel(input_embeds, query_embeds, mask, label,
           wq_w, wq_b, wk_w, wk_b, wv_w, wv_b, wo_w, wo_b, **_kw):
    del label
    input_embeds = np.asarray(input_embeds, np.float32)
    query_embeds = np.asarray(query_embeds, np.float32)
    mask = np.asarray(mask, np.float32)
    args = [np.asarray(a, np.float32) for a in
            (wq_w, wq_b, wk_w, wk_b, wv_w, wv_b, wo_w)]
    wo_b = np.asarray(wo_b, np.float32)

    if "nc" not in _CACHE:
        _CACHE["nc"] = build_bass()
    nc = _CACHE["nc"]

    in_maps = _prep_inputs(input_embeds, query_embeds, mask, *args)
    res = run_bass_kernel_spmd(nc, in_maps, core_ids=list(range(8)))
    results = res.results

    # attn: [b, g, h, k, q] -> [B, H, k, q] -> transposed view [B, H, q, k]
    attnT_full = np.stack(
        [results[core]["attnT"] for core in range(8)]
    ).reshape(B, H, S, S)
    attn = attnT_full.swapaxes(2, 3)

    out = np.stack(
        [results[b * HG]["outp"] + results[b * HG + 1]["outp"] + wo_b
         for b in range(B)]
    )
    return out, attn
